# revision 19
# baseline (speedup 1.0000x reference)
"""Trainium2 Bass kernel for nn_LowFreqPenaltyLoss.

Computes mean(|einsum('ih,nchw,jw->ncij', Ch, delta, Cw)|) for
delta [256, 3, 256, 256] f32, Ch/Cw the 8x256 unnormalized DCT-II bases.

Strategy (data-parallel over batch, 8 cores), per core 96 images = 24 MiB:

  LOAD (the problem is HBM-stream-bound): 11 groups of 8 images (2 MiB)
  + 2 tail groups of 4 images (1 MiB), all via SWDGE DMAs that cast
  f32->bf16 inline.  Partition p receives a CONTIGUOUS HBM chunk (16-17 KiB)
  so descriptors/packets are full-sized (the old per-row 1 KiB gather
  streamed at ~275 GB/s read-side; flat chunks reach ~400-410 GB/s).

  ENGINE-15 SHED: SDMA engine 15 is intermittently ~15-25% slower per
  packet (known SWDGE descriptor-ring port contention); in those runs it
  alone stretches the stream by ~10 us (engines are partition-pinned, no
  work stealing).  Main groups therefore give partitions 120-127 only 1 of
  the 2048 group rows (two rectangular DMAs: [0:120]x17 rows + [120:128]x1
  row), halving engine 15's and 13's load at +6% on the rest: the slow mode
  stops being the long pole at a ~2 us cost in the fast mode.

  STAGE A (contract h): each partition's rows sit on the free axis, so the
  DCT-H contraction is 17 accumulating matmuls with block-sparse weights
  wa17[p, r, 8q+i] = Ch[i, h] for global row 17p+r = 256q + h (r=0 uses all
  128 partitions, r>=1 only [0:120)) -> psumA[8q+i, w] f32.

  STAGE B (contract w): ACT copies psumA -> SBUF (casts bf16), 2 PE
  transposes of [64,128] chunks (each into its own PSUM bank), DVE copies
  out, matmul with CwT -> ps2[j, (q,i)], fused |.|+sum on DVE into acc[8,1].
  Stage B of group g-1 is emitted AFTER stage A of group g so the PE (which
  runs in program order) never stalls mid-stream on ACT/DVE round-trips.

  FINISH: the 8 per-partition partials ship via a Scalar-engine DMA (it
  overlaps the SP exit-drain sem walk); the host sums 8 cores x 8 partials
  and divides by 49152.  bf16 inputs + f32 PSUM accumulation give ~2e-4
  relative error on the final scalar (gate is 2e-2).
"""

import sys
import types

for _p in ("/root/.axon_site/_ro/trn_rl_repo", "/opt/trn_rl_repo"):
    if _p not in sys.path:
        sys.path.append(_p)

import numpy as np
from contextlib import ExitStack

import concourse.bass as bass
import concourse.tile as tile
from concourse import mybir, bass_utils
from concourse._compat import with_exitstack
from concourse.vector_clock import ScopedClock

# ---------------------------------------------------------------------------
# Workarounds for this image.
# ---------------------------------------------------------------------------

# walrus on this image rejects >1 sync-wait on one CTRL instruction; split the
# Tile exit-drain's waits across follow-up nops (same engine, program order).
# Also: the stock tail (barrier + per-sem clear + barrier) costs ~8-10us of
# EVSEM butterfly at kernel end. The kernel is one-shot per NEFF execution and
# NRT re-initialises semaphores per execution, so keep only the drain + DMA
# completion waits.
_ORIG_DAB = tile.TileContext._drain_and_barrier
_USE_STOCK_TAIL = False


def _patched_drain_and_barrier(self, tick_clock, wait_clock):
    if _USE_STOCK_TAIL:
        return _ORIG_DAB(self, tick_clock, wait_clock)
    nc = self.nc
    drain_inst = nc.sync.drain()
    wait_clock.add_sem_waits(
        drain_inst.ins, ScopedClock({None: tick_clock.global_clock})
    )
    si = drain_inst.ins.sync_info
    waits = list(si.on_wait) if si and si.on_wait else []
    if len(waits) > 1:
        drain_inst.ins.sync_info = mybir.SyncInfo(
            on_wait=[waits[0]], on_update=list(si.on_update or [])
        )
        for w in waits[1:]:
            nop = nc.sync.nop(nofuse=True, hint="drain_wait_split")
            nop.ins.sync_info = mybir.SyncInfo(on_wait=[w], on_update=[])
    popped = nc._tile_sem_poison_stack.pop()
    assert popped is self._sem_poison


tile.TileContext._drain_and_barrier = _patched_drain_and_barrier

# zero-egress container: profiling artifact upload must stay local.
bass_utils.upload_artifacts = lambda d: d


def _strip_main_barrier(nc):
    """Drop the prologue all-engine barrier AND the dead const memsets in
    'main': the barrier's only role is to fence the framework preamble (dead
    const memsets + per-engine table loads) from the kernel, but per-engine
    program order already covers the table loads, and nothing reads the
    const tiles (verified: no instruction references const-* memrefs).  Each
    engine then branches into the kernel as soon as its own init finishes
    instead of waiting for the slowest engine (~1.4us of startup)."""
    for fn in nc.m.functions:
        for bb in fn.blocks:
            if bb.name != "main":
                continue
            bb.instructions[:] = [
                i for i in bb.instructions
                if not isinstance(
                    i,
                    (mybir.InstEventSemaphore, mybir.InstDrain, mybir.InstMemset),
                )
            ]


def _split_multi_waits(nc):
    """walrus on this image rejects >1 sync-wait per instruction: hoist extra
    waits onto fresh NoOps inserted just before, on the same engine."""
    for fn in nc.m.functions:
        for bb in fn.blocks:
            new_insts = []
            for inst in bb.instructions:
                si = inst.sync_info
                waits = list(si.on_wait) if si and si.on_wait else []
                if len(waits) > 1:
                    for w in waits[:-1]:
                        nop = mybir.InstNoOp(
                            name=nc.get_next_instruction_name(),
                            sync_info=mybir.SyncInfo(on_wait=[w], on_update=[]),
                            bass_nofuse=True,
                            engine=inst.engine,
                        )
                        new_insts.append(nop)
                    inst.sync_info = mybir.SyncInfo(
                        on_wait=[waits[-1]], on_update=list(si.on_update or [])
                    )
                new_insts.append(inst)
            bb.instructions[:] = new_insts

# ---------------------------------------------------------------------------
# Problem constants (hardcoded; kernel.py must be self-contained).
# ---------------------------------------------------------------------------

B, C, H, W = 256, 3, 256, 256
LOW_A = LOW_B = 8
N_CORES = 8
IMGS_PER_CORE = (B // N_CORES) * C          # 96
GRP = 8                                     # images per main group (2 MiB f32)
N_MAIN = IMGS_PER_CORE // GRP - 1           # 11 main groups; 2x4-img tails
GROWS = GRP * H                             # 2048 rows per main group
SHED_ROWS = 17                              # rows per full partition (120x17+8x1)
TOTAL_LOW = B * C * LOW_A * LOW_B           # 49152 -> mean divisor

F32 = mybir.dt.float32
BF16 = mybir.dt.bfloat16


def _dct_basis(K, N):
    n = np.arange(N, dtype=np.float64)
    k = np.arange(K, dtype=np.float64)
    return (2.0 * np.cos(np.pi * (2.0 * n[None, :] + 1.0) * k[:, None] / (2.0 * N))).astype(
        np.float32
    )


def _make_consts():
    Ch = _dct_basis(LOW_A, H)   # [8, 256]
    Cw = _dct_basis(LOW_B, W)   # [8, 256]
    # Shed layout weights: partition p < 120 holds group rows 17p..17p+16,
    # partitions 120..127 hold rows 2040..2047 (one each).  Global row
    # g = 256q + h -> out row 8q+i gets Ch[i, h].
    wa17 = np.zeros((128, SHED_ROWS, 64), np.float32)
    for p in range(120):
        for r in range(SHED_ROWS):
            g = SHED_ROWS * p + r
            q, h = divmod(g, H)
            wa17[p, r, 8 * q:8 * q + 8] = Ch[:, h]
    for p in range(120, 128):
        q, h = divmod(2040 + (p - 120), H)
        wa17[p, 0, 8 * q:8 * q + 8] = Ch[:, h]
    # 4-image tail groups (uniform): image q = p//32, rows h = 8*(p%32) + r.
    wa4 = np.zeros((128, 8, 32), np.float32)
    for p in range(128):
        q, pp = p // 32, p % 32
        for r in range(8):
            wa4[p, r, 8 * q:8 * q + 8] = Ch[:, 8 * pp + r]
    # cwt[p, wc, j] = Cw[j, wc*128+p]
    cwt = np.zeros((128, 2, LOW_B), np.float32)
    for wc in range(2):
        cwt[:, wc, :] = Cw[:, wc * 128:(wc + 1) * 128].T
    import ml_dtypes
    bf16 = ml_dtypes.bfloat16
    ident = np.eye(128, dtype=bf16)
    return wa17.astype(bf16), wa4.astype(bf16), cwt.astype(bf16), ident


WA17, WA4, CWT, IDENT = _make_consts()


# ---------------------------------------------------------------------------
# Kernel body (per core; SPMD over 8 cores).
# ---------------------------------------------------------------------------

@with_exitstack
def _lowfreq_kernel(ctx: ExitStack, tc, out_ap, delta_ap, wa17_ap, wa4_ap,
                    cwt_ap, ident_ap):
    nc = tc.nc

    const_pool = ctx.enter_context(tc.tile_pool(name="const", bufs=1))
    in8_pool = ctx.enter_context(tc.tile_pool(name="in8", bufs=N_MAIN))
    in4_pool = ctx.enter_context(tc.tile_pool(name="in4", bufs=2))
    sS_pool = ctx.enter_context(tc.tile_pool(name="sS", bufs=3))
    tS_pool = ctx.enter_context(tc.tile_pool(name="tS", bufs=3))
    red_pool = ctx.enter_context(tc.tile_pool(name="red", bufs=2))
    acc_pool = ctx.enter_context(tc.tile_pool(name="acc", bufs=1))
    psA_pool = ctx.enter_context(tc.tile_pool(name="psA", bufs=3, space="PSUM"))
    psT_pool = ctx.enter_context(tc.tile_pool(name="psT", bufs=3, space="PSUM"))
    ps2_pool = ctx.enter_context(tc.tile_pool(name="ps2", bufs=2, space="PSUM"))

    # constants (HWDGE/Sync queue; lands well before first compute)
    wa17 = const_pool.tile([128, SHED_ROWS, 64], BF16)
    nc.sync.dma_start(wa17[:], wa17_ap)
    wa4 = const_pool.tile([128, 8, 32], BF16)
    nc.sync.dma_start(wa4[:], wa4_ap)
    cwt = const_pool.tile([128, 2, LOW_B], BF16)
    nc.sync.dma_start(cwt[:], cwt_ap)
    ident = const_pool.tile([128, 128], BF16)
    nc.sync.dma_start(ident[:], ident_ap)

    acc = acc_pool.tile([8, 1], F32)
    nc.vector.memset(acc[:], 0.0)

    # issue ALL input DMAs upfront (SWDGE, f32->bf16 inline cast).
    subs = []
    for g in range(N_MAIN):
        gt = in8_pool.tile([128, SHED_ROWS, 256], BF16, tag="gt8")
        fl = delta_ap[GRP * g:GRP * g + GRP].rearrange("q h w -> (q h) w")
        nc.gpsimd.dma_start(
            gt[0:120, :, :],
            fl[0:120 * SHED_ROWS].rearrange("(p r) w -> p (r w)", p=120,
                                            r=SHED_ROWS),
        )
        nc.gpsimd.dma_start(gt[120:128, 0, :], fl[120 * SHED_ROWS:GROWS])
        subs.append((gt, "shed", GRP))
    for t in range(2):
        gt4 = in4_pool.tile([128, 8, 256], BF16, tag="gt4")
        src = delta_ap[GRP * N_MAIN + 4 * t:GRP * N_MAIN + 4 * t + 4]
        nc.gpsimd.dma_start(
            gt4[:],
            src.rearrange("q (pp r) w -> (q pp) (r w)", pp=32, r=8),
        )
        subs.append((gt4, "tail", 4))

    def stage_a(sub):
        gt, kind, n_img = sub
        n_out = 8 * n_img
        psumA = psA_pool.tile([n_out, 256], F32, tag="psA")
        if kind == "shed":
            for r in range(SHED_ROWS):
                if r == 0:
                    nc.tensor.matmul(
                        psumA[:], lhsT=wa17[:, 0, :], rhs=gt[:, 0, :],
                        start=True, stop=False,
                    )
                else:
                    nc.tensor.matmul(
                        psumA[:], lhsT=wa17[0:120, r, :], rhs=gt[0:120, r, :],
                        start=False, stop=(r == SHED_ROWS - 1),
                    )
        else:
            for r in range(8):
                nc.tensor.matmul(
                    psumA[:], lhsT=wa4[:, r, :], rhs=gt[:, r, :],
                    start=(r == 0), stop=(r == 7),
                )
        # PSUM -> SBUF with f32->bf16 cast (ACT engine; off the PE timeline)
        sA = sS_pool.tile([n_out, 256], BF16, tag="sA")
        nc.scalar.copy(sA[:], psumA[:])
        return sA, n_out

    def stage_b(sA, n_out):
        # 2 PE transposes (own PSUM tiles: transpose-mode output must start
        # at a bank boundary on HW), DVE copies out, then contract w into
        # ps2[j, (q,i)], fused |.|+sum, accumulate.
        tS = tS_pool.tile([128, 2, n_out], BF16, tag="tS")
        for wc in range(2):
            tp = psT_pool.tile([128, n_out], BF16, tag="tp")
            nc.tensor.transpose(
                tp[:],
                sA[:, 128 * wc:128 * wc + 128],
                ident[0:n_out, 0:n_out],
            )
            nc.vector.tensor_copy(tS[:, wc, :], tp[:])
        ps2 = ps2_pool.tile([8, n_out], F32, tag="ps2")
        for wc in range(2):
            nc.tensor.matmul(
                ps2[:],
                lhsT=cwt[:, wc, :],
                rhs=tS[:, wc, :],
                start=(wc == 0),
                stop=(wc == 1),
            )
        red = red_pool.tile([8, 1], F32)
        nc.vector.tensor_reduce(
            red[:], ps2[:], axis=mybir.AxisListType.X,
            op=mybir.AluOpType.add, apply_absolute_value=True,
        )
        nc.vector.tensor_add(acc[:], acc[:], red[:])

    # Software pipeline: emit stage B of group g-1 AFTER stage A of group g,
    # so the PE (which executes in program order) never stalls mid-stream on
    # the ACT/DVE round-trips of stage B.
    prev = None
    for sub in subs:
        cur = stage_a(sub)
        if prev is not None:
            stage_b(*prev)
        prev = cur
    stage_b(*prev)

    # ship the 8 per-partition partials; the host does the final sum + mean.
    # Issue on Scalar (HWDGE) so it overlaps the SP exit-drain sem walk.
    nc.scalar.dma_start(out_ap, acc[:])


# ---------------------------------------------------------------------------
# Build + run.
# ---------------------------------------------------------------------------

_CACHED_NC = None


def _build(for_sim=False):
    global _CACHED_NC, _USE_STOCK_TAIL
    if not for_sim and _CACHED_NC is not None:
        return _CACHED_NC
    _USE_STOCK_TAIL = for_sim
    nc = bass.Bass("TRN2", target_bir_lowering=False, debug=False)
    delta = nc.dram_tensor("delta", [IMGS_PER_CORE, H, W], F32, kind="ExternalInput")
    wa17 = nc.dram_tensor("wa17", list(WA17.shape), BF16, kind="ExternalInput")
    wa4 = nc.dram_tensor("wa4", list(WA4.shape), BF16, kind="ExternalInput")
    cwt = nc.dram_tensor("cwt", list(CWT.shape), BF16, kind="ExternalInput")
    ident = nc.dram_tensor("ident", list(IDENT.shape), BF16, kind="ExternalInput")
    out = nc.dram_tensor("out", [8, 1], F32, kind="ExternalOutput")

    with tile.TileContext(nc) as tc:
        _lowfreq_kernel(
            tc, out.ap(), delta.ap(), wa17.ap(), wa4.ap(), cwt.ap(), ident.ap()
        )
    _USE_STOCK_TAIL = False
    if for_sim:
        return nc
    _strip_main_barrier(nc)
    _split_multi_waits(nc)
    _CACHED_NC = nc
    return nc


def _run(delta, **spmd_kwargs):
    import os
    os.environ["JAX_PLATFORMS"] = "axon"   # harness may have pinned cpu for the reference
    nc = _build()
    delta = np.ascontiguousarray(np.asarray(delta, dtype=np.float32))
    assert delta.shape == (B, C, H, W)
    shards = delta.reshape(N_CORES, IMGS_PER_CORE, H, W)
    in_maps = [
        {
            "delta": shards[i],
            "wa17": WA17,
            "wa4": WA4,
            "cwt": CWT,
            "ident": IDENT,
        }
        for i in range(N_CORES)
    ]
    try:
        res = bass_utils.run_bass_kernel_spmd(
            nc, in_maps, core_ids=list(range(N_CORES)), **spmd_kwargs
        )
    except Exception:
        # transient NRT_EXEC_UNIT_UNRECOVERABLE has been observed on this
        # terminal; one retry typically succeeds.
        res = bass_utils.run_bass_kernel_spmd(
            nc, in_maps, core_ids=list(range(N_CORES)), **spmd_kwargs
        )
    total = np.float64(0.0)
    for r in res.results:
        total += np.asarray(r["out"], np.float64).sum()
    return np.float32(total / TOTAL_LOW).reshape(()), res


def kernel(delta):
    out, _ = _run(delta)
    return out


# revision 20
# speedup vs baseline: 1.0186x; 1.0186x over previous
"""Trainium2 Bass kernel for nn_LowFreqPenaltyLoss.

Computes mean(|einsum('ih,nchw,jw->ncij', Ch, delta, Cw)|) for
delta [256, 3, 256, 256] f32, Ch/Cw the 8x256 unnormalized DCT-II bases.

Strategy (data-parallel over batch, 8 cores), per core 96 images = 24 MiB:

  LOAD (the problem is HBM-stream-bound): 11 groups of 8 images (2 MiB)
  + 2 tail groups of 4 images (1 MiB), all via SWDGE DMAs that cast
  f32->bf16 inline.  Partition p receives a CONTIGUOUS HBM chunk (16-17 KiB)
  so descriptors/packets are full-sized (the old per-row 1 KiB gather
  streamed at ~275 GB/s read-side; flat chunks reach ~400-410 GB/s).

  ENGINE-15 SHED: SDMA engine 15 is intermittently ~15-25% slower per
  packet (known SWDGE descriptor-ring port contention); in those runs it
  alone stretches the stream by ~10 us (engines are partition-pinned, no
  work stealing).  Main groups therefore give partitions 120-127 only 1 of
  the 2048 group rows (two rectangular DMAs: [0:120]x17 rows + [120:128]x1
  row), halving engine 15's and 13's load at +6% on the rest: the slow mode
  stops being the long pole at a ~2 us cost in the fast mode.

  STAGE A (contract h): each partition's rows sit on the free axis, so the
  DCT-H contraction is 17 accumulating matmuls with block-sparse weights
  wa17[p, r, 8q+i] = Ch[i, h] for global row 17p+r = 256q + h (r=0 uses all
  128 partitions, r>=1 only [0:120)) -> psumA[8q+i, w] f32.

  STAGE B (contract w): ACT copies psumA -> SBUF (casts bf16), 2 PE
  transposes of [64,128] chunks (each into its own PSUM bank), DVE copies
  out, matmul with CwT -> ps2[j, (q,i)], fused |.|+sum on DVE into acc[8,1].
  Stage B of group g-1 is emitted AFTER stage A of group g so the PE (which
  runs in program order) never stalls mid-stream on ACT/DVE round-trips.

  FINISH: the 8 per-partition partials ship via a Scalar-engine DMA (it
  overlaps the SP exit-drain sem walk); the host sums 8 cores x 8 partials
  and divides by 49152.  bf16 inputs + f32 PSUM accumulation give ~2e-4
  relative error on the final scalar (gate is 2e-2).
"""

import sys
import types

for _p in ("/root/.axon_site/_ro/trn_rl_repo", "/opt/trn_rl_repo"):
    if _p not in sys.path:
        sys.path.append(_p)

import numpy as np
from contextlib import ExitStack

import concourse.bass as bass
import concourse.tile as tile
from concourse import mybir, bass_utils
from concourse._compat import with_exitstack
from concourse.vector_clock import ScopedClock

# ---------------------------------------------------------------------------
# Workarounds for this image.
# ---------------------------------------------------------------------------

# walrus on this image rejects >1 sync-wait on one CTRL instruction; split the
# Tile exit-drain's waits across follow-up nops (same engine, program order).
# Also: the stock tail (barrier + per-sem clear + barrier) costs ~8-10us of
# EVSEM butterfly at kernel end. The kernel is one-shot per NEFF execution and
# NRT re-initialises semaphores per execution, so keep only the drain + DMA
# completion waits.
_ORIG_DAB = tile.TileContext._drain_and_barrier
_USE_STOCK_TAIL = False


def _patched_drain_and_barrier(self, tick_clock, wait_clock):
    if _USE_STOCK_TAIL:
        return _ORIG_DAB(self, tick_clock, wait_clock)
    nc = self.nc
    drain_inst = nc.sync.drain()
    wait_clock.add_sem_waits(
        drain_inst.ins, ScopedClock({None: tick_clock.global_clock})
    )
    si = drain_inst.ins.sync_info
    waits = list(si.on_wait) if si and si.on_wait else []
    if len(waits) > 1:
        drain_inst.ins.sync_info = mybir.SyncInfo(
            on_wait=[waits[0]], on_update=list(si.on_update or [])
        )
        for w in waits[1:]:
            nop = nc.sync.nop(nofuse=True, hint="drain_wait_split")
            nop.ins.sync_info = mybir.SyncInfo(on_wait=[w], on_update=[])
    popped = nc._tile_sem_poison_stack.pop()
    assert popped is self._sem_poison


tile.TileContext._drain_and_barrier = _patched_drain_and_barrier

# zero-egress container: profiling artifact upload must stay local.
bass_utils.upload_artifacts = lambda d: d


def _strip_main_barrier(nc):
    """Drop the prologue all-engine barrier AND the dead const memsets in
    'main': the barrier's only role is to fence the framework preamble (dead
    const memsets + per-engine table loads) from the kernel, but per-engine
    program order already covers the table loads, and nothing reads the
    const tiles (verified: no instruction references const-* memrefs).  Each
    engine then branches into the kernel as soon as its own init finishes
    instead of waiting for the slowest engine (~1.4us of startup)."""
    for fn in nc.m.functions:
        for bb in fn.blocks:
            if bb.name != "main":
                continue
            bb.instructions[:] = [
                i for i in bb.instructions
                if not isinstance(
                    i,
                    (mybir.InstEventSemaphore, mybir.InstDrain, mybir.InstMemset),
                )
            ]


def _split_multi_waits(nc):
    """walrus on this image rejects >1 sync-wait per instruction: hoist extra
    waits onto fresh NoOps inserted just before, on the same engine."""
    for fn in nc.m.functions:
        for bb in fn.blocks:
            new_insts = []
            for inst in bb.instructions:
                si = inst.sync_info
                waits = list(si.on_wait) if si and si.on_wait else []
                if len(waits) > 1:
                    for w in waits[:-1]:
                        nop = mybir.InstNoOp(
                            name=nc.get_next_instruction_name(),
                            sync_info=mybir.SyncInfo(on_wait=[w], on_update=[]),
                            bass_nofuse=True,
                            engine=inst.engine,
                        )
                        new_insts.append(nop)
                    inst.sync_info = mybir.SyncInfo(
                        on_wait=[waits[-1]], on_update=list(si.on_update or [])
                    )
                new_insts.append(inst)
            bb.instructions[:] = new_insts

# ---------------------------------------------------------------------------
# Problem constants (hardcoded; kernel.py must be self-contained).
# ---------------------------------------------------------------------------

import os as _os
SHED = _os.environ.get("KSHED", "1") == "1"

B, C, H, W = 256, 3, 256, 256
LOW_A = LOW_B = 8
N_CORES = 8
IMGS_PER_CORE = (B // N_CORES) * C          # 96
GRP = 8                                     # images per main group (2 MiB f32)
N_MAIN = IMGS_PER_CORE // GRP - 1           # 11 main groups; 2x4-img tails
GROWS = GRP * H                             # 2048 rows per main group
SHED_ROWS = 17 if SHED else 16              # rows per full partition
TOTAL_LOW = B * C * LOW_A * LOW_B           # 49152 -> mean divisor

F32 = mybir.dt.float32
BF16 = mybir.dt.bfloat16


def _dct_basis(K, N):
    n = np.arange(N, dtype=np.float64)
    k = np.arange(K, dtype=np.float64)
    return (2.0 * np.cos(np.pi * (2.0 * n[None, :] + 1.0) * k[:, None] / (2.0 * N))).astype(
        np.float32
    )


def _make_consts():
    Ch = _dct_basis(LOW_A, H)   # [8, 256]
    Cw = _dct_basis(LOW_B, W)   # [8, 256]
    # Shed layout weights: partition p < 120 holds group rows 17p..17p+16,
    # partitions 120..127 hold rows 2040..2047 (one each).  Global row
    # g = 256q + h -> out row 8q+i gets Ch[i, h].
    wa17 = np.zeros((128, SHED_ROWS, 64), np.float32)
    if SHED:
        for p in range(120):
            for r in range(SHED_ROWS):
                g = SHED_ROWS * p + r
                q, h = divmod(g, H)
                wa17[p, r, 8 * q:8 * q + 8] = Ch[:, h]
        for p in range(120, 128):
            q, h = divmod(2040 + (p - 120), H)
            wa17[p, 0, 8 * q:8 * q + 8] = Ch[:, h]
    else:
        for p in range(128):
            for r in range(SHED_ROWS):
                g = SHED_ROWS * p + r
                q, h = divmod(g, H)
                wa17[p, r, 8 * q:8 * q + 8] = Ch[:, h]
    # 4-image tail groups (uniform): image q = p//32, rows h = 8*(p%32) + r.
    wa4 = np.zeros((128, 8, 32), np.float32)
    for p in range(128):
        q, pp = p // 32, p % 32
        for r in range(8):
            wa4[p, r, 8 * q:8 * q + 8] = Ch[:, 8 * pp + r]
    # cwt[p, wc, j] = Cw[j, wc*128+p]
    cwt = np.zeros((128, 2, LOW_B), np.float32)
    for wc in range(2):
        cwt[:, wc, :] = Cw[:, wc * 128:(wc + 1) * 128].T
    import ml_dtypes
    bf16 = ml_dtypes.bfloat16
    ident = np.eye(128, dtype=bf16)
    return wa17.astype(bf16), wa4.astype(bf16), cwt.astype(bf16), ident


WA17, WA4, CWT, IDENT = _make_consts()


# ---------------------------------------------------------------------------
# Kernel body (per core; SPMD over 8 cores).
# ---------------------------------------------------------------------------

@with_exitstack
def _lowfreq_kernel(ctx: ExitStack, tc, out_ap, delta_ap, wa17_ap, wa4_ap,
                    cwt_ap, ident_ap):
    nc = tc.nc

    const_pool = ctx.enter_context(tc.tile_pool(name="const", bufs=1))
    in8_pool = ctx.enter_context(tc.tile_pool(name="in8", bufs=N_MAIN))
    in4_pool = ctx.enter_context(tc.tile_pool(name="in4", bufs=2))
    sS_pool = ctx.enter_context(tc.tile_pool(name="sS", bufs=3))
    tS_pool = ctx.enter_context(tc.tile_pool(name="tS", bufs=3))
    red_pool = ctx.enter_context(tc.tile_pool(name="red", bufs=2))
    acc_pool = ctx.enter_context(tc.tile_pool(name="acc", bufs=1))
    psA_pool = ctx.enter_context(tc.tile_pool(name="psA", bufs=3, space="PSUM"))
    psT_pool = ctx.enter_context(tc.tile_pool(name="psT", bufs=3, space="PSUM"))
    ps2_pool = ctx.enter_context(tc.tile_pool(name="ps2", bufs=2, space="PSUM"))

    # constants (HWDGE/Sync queue; lands well before first compute)
    wa17 = const_pool.tile([128, SHED_ROWS, 64], BF16)
    nc.sync.dma_start(wa17[:], wa17_ap)
    wa4 = const_pool.tile([128, 8, 32], BF16)
    nc.sync.dma_start(wa4[:], wa4_ap)
    cwt = const_pool.tile([128, 2, LOW_B], BF16)
    nc.sync.dma_start(cwt[:], cwt_ap)
    ident = const_pool.tile([128, 128], BF16)
    nc.sync.dma_start(ident[:], ident_ap)

    acc = acc_pool.tile([8, 1], F32)
    nc.vector.memset(acc[:], 0.0)

    # issue ALL input DMAs upfront (SWDGE, f32->bf16 inline cast).
    subs = []
    for g in range(N_MAIN):
        gt = in8_pool.tile([128, SHED_ROWS, 256], BF16, tag="gt8")
        fl = delta_ap[GRP * g:GRP * g + GRP].rearrange("q h w -> (q h) w")
        if SHED:
            nc.gpsimd.dma_start(
                gt[0:120, :, :],
                fl[0:120 * SHED_ROWS].rearrange("(p r) w -> p (r w)", p=120,
                                                r=SHED_ROWS),
            )
            nc.gpsimd.dma_start(gt[120:128, 0, :], fl[120 * SHED_ROWS:GROWS])
        else:
            nc.gpsimd.dma_start(
                gt[:],
                fl.rearrange("(p r) w -> p (r w)", p=128, r=SHED_ROWS),
            )
        subs.append((gt, "shed", GRP))
    for t in range(2):
        gt4 = in4_pool.tile([128, 8, 256], BF16, tag="gt4")
        src = delta_ap[GRP * N_MAIN + 4 * t:GRP * N_MAIN + 4 * t + 4]
        nc.gpsimd.dma_start(
            gt4[:],
            src.rearrange("q (pp r) w -> (q pp) (r w)", pp=32, r=8),
        )
        subs.append((gt4, "tail", 4))

    def stage_a(sub):
        gt, kind, n_img = sub
        n_out = 8 * n_img
        psumA = psA_pool.tile([n_out, 256], F32, tag="psA")
        if kind == "shed":
            for r in range(SHED_ROWS):
                if r == 0 or not SHED:
                    nc.tensor.matmul(
                        psumA[:], lhsT=wa17[:, r, :], rhs=gt[:, r, :],
                        start=(r == 0), stop=(r == SHED_ROWS - 1),
                    )
                else:
                    nc.tensor.matmul(
                        psumA[:], lhsT=wa17[0:120, r, :], rhs=gt[0:120, r, :],
                        start=False, stop=(r == SHED_ROWS - 1),
                    )
        else:
            for r in range(8):
                nc.tensor.matmul(
                    psumA[:], lhsT=wa4[:, r, :], rhs=gt[:, r, :],
                    start=(r == 0), stop=(r == 7),
                )
        # PSUM -> SBUF with f32->bf16 cast (ACT engine; off the PE timeline)
        sA = sS_pool.tile([n_out, 256], BF16, tag="sA")
        nc.scalar.copy(sA[:], psumA[:])
        return sA, n_out

    def stage_b(sA, n_out):
        # 2 PE transposes (own PSUM tiles: transpose-mode output must start
        # at a bank boundary on HW), DVE copies out, then contract w into
        # ps2[j, (q,i)], fused |.|+sum, accumulate.
        tS = tS_pool.tile([128, 2, n_out], BF16, tag="tS")
        for wc in range(2):
            tp = psT_pool.tile([128, n_out], BF16, tag="tp")
            nc.tensor.transpose(
                tp[:],
                sA[:, 128 * wc:128 * wc + 128],
                ident[0:n_out, 0:n_out],
            )
            nc.vector.tensor_copy(tS[:, wc, :], tp[:])
        ps2 = ps2_pool.tile([8, n_out], F32, tag="ps2")
        for wc in range(2):
            nc.tensor.matmul(
                ps2[:],
                lhsT=cwt[:, wc, :],
                rhs=tS[:, wc, :],
                start=(wc == 0),
                stop=(wc == 1),
            )
        red = red_pool.tile([8, 1], F32)
        nc.vector.tensor_reduce(
            red[:], ps2[:], axis=mybir.AxisListType.X,
            op=mybir.AluOpType.add, apply_absolute_value=True,
        )
        nc.vector.tensor_add(acc[:], acc[:], red[:])

    # Software pipeline: emit stage B of group g-1 AFTER stage A of group g,
    # so the PE (which executes in program order) never stalls mid-stream on
    # the ACT/DVE round-trips of stage B.
    prev = None
    for sub in subs:
        cur = stage_a(sub)
        if prev is not None:
            stage_b(*prev)
        prev = cur
    stage_b(*prev)

    # ship the 8 per-partition partials; the host does the final sum + mean.
    # Issue on Scalar (HWDGE) so it overlaps the SP exit-drain sem walk.
    nc.scalar.dma_start(out_ap, acc[:])


# ---------------------------------------------------------------------------
# Build + run.
# ---------------------------------------------------------------------------

_CACHED_NC = None


def _build(for_sim=False):
    global _CACHED_NC, _USE_STOCK_TAIL
    if not for_sim and _CACHED_NC is not None:
        return _CACHED_NC
    _USE_STOCK_TAIL = for_sim
    nc = bass.Bass("TRN2", target_bir_lowering=False, debug=False)
    delta = nc.dram_tensor("delta", [IMGS_PER_CORE, H, W], F32, kind="ExternalInput")
    wa17 = nc.dram_tensor("wa17", list(WA17.shape), BF16, kind="ExternalInput")
    wa4 = nc.dram_tensor("wa4", list(WA4.shape), BF16, kind="ExternalInput")
    cwt = nc.dram_tensor("cwt", list(CWT.shape), BF16, kind="ExternalInput")
    ident = nc.dram_tensor("ident", list(IDENT.shape), BF16, kind="ExternalInput")
    out = nc.dram_tensor("out", [8, 1], F32, kind="ExternalOutput")

    with tile.TileContext(nc) as tc:
        _lowfreq_kernel(
            tc, out.ap(), delta.ap(), wa17.ap(), wa4.ap(), cwt.ap(), ident.ap()
        )
    _USE_STOCK_TAIL = False
    if for_sim:
        return nc
    _strip_main_barrier(nc)
    _split_multi_waits(nc)
    _CACHED_NC = nc
    return nc


def _run(delta, **spmd_kwargs):
    import os
    os.environ["JAX_PLATFORMS"] = "axon"   # harness may have pinned cpu for the reference
    nc = _build()
    delta = np.ascontiguousarray(np.asarray(delta, dtype=np.float32))
    assert delta.shape == (B, C, H, W)
    shards = delta.reshape(N_CORES, IMGS_PER_CORE, H, W)
    in_maps = [
        {
            "delta": shards[i],
            "wa17": WA17,
            "wa4": WA4,
            "cwt": CWT,
            "ident": IDENT,
        }
        for i in range(N_CORES)
    ]
    try:
        res = bass_utils.run_bass_kernel_spmd(
            nc, in_maps, core_ids=list(range(N_CORES)), **spmd_kwargs
        )
    except Exception:
        # transient NRT_EXEC_UNIT_UNRECOVERABLE has been observed on this
        # terminal; one retry typically succeeds.
        res = bass_utils.run_bass_kernel_spmd(
            nc, in_maps, core_ids=list(range(N_CORES)), **spmd_kwargs
        )
    total = np.float64(0.0)
    for r in res.results:
        total += np.asarray(r["out"], np.float64).sum()
    return np.float32(total / TOTAL_LOW).reshape(()), res


def kernel(delta):
    out, _ = _run(delta)
    return out


# revision 21
# speedup vs baseline: 1.1018x; 1.0817x over previous
"""Trainium2 Bass kernel for nn_LowFreqPenaltyLoss.

Computes mean(|einsum('ih,nchw,jw->ncij', Ch, delta, Cw)|) for
delta [256, 3, 256, 256] f32, Ch/Cw the 8x256 unnormalized DCT-II bases.

Strategy (data-parallel over batch, 8 cores), per core 96 images = 24 MiB:

  LOAD (the problem is HBM-stream-bound): 11 groups of 8 images (2 MiB)
  + 2 tail groups of 4 images (1 MiB), all via SWDGE DMAs that cast
  f32->bf16 inline.  Partition p receives a CONTIGUOUS HBM chunk (16-17 KiB)
  so descriptors/packets are full-sized (the old per-row 1 KiB gather
  streamed at ~275 GB/s read-side; flat chunks reach ~400-410 GB/s).

  ENGINE-15 SHED: SDMA engine 15 is intermittently ~15-25% slower per
  packet (known SWDGE descriptor-ring port contention); in those runs it
  alone stretches the stream by ~10 us (engines are partition-pinned, no
  work stealing).  Main groups therefore give partitions 120-127 only 1 of
  the 2048 group rows (two rectangular DMAs: [0:120]x17 rows + [120:128]x1
  row), halving engine 15's and 13's load at +6% on the rest: the slow mode
  stops being the long pole at a ~2 us cost in the fast mode.

  STAGE A (contract h): each partition's rows sit on the free axis, so the
  DCT-H contraction is 17 accumulating matmuls with block-sparse weights
  wa17[p, r, 8q+i] = Ch[i, h] for global row 17p+r = 256q + h (r=0 uses all
  128 partitions, r>=1 only [0:120)) -> psumA[8q+i, w] f32.

  STAGE B (contract w): ACT copies psumA -> SBUF (casts bf16), 2 PE
  transposes of [64,128] chunks (each into its own PSUM bank), DVE copies
  out, matmul with CwT -> ps2[j, (q,i)], fused |.|+sum on DVE into acc[8,1].
  Stage B of group g-1 is emitted AFTER stage A of group g so the PE (which
  runs in program order) never stalls mid-stream on ACT/DVE round-trips.

  FINISH: the 8 per-partition partials ship via a Scalar-engine DMA (it
  overlaps the SP exit-drain sem walk); the host sums 8 cores x 8 partials
  and divides by 49152.  bf16 inputs + f32 PSUM accumulation give ~2e-4
  relative error on the final scalar (gate is 2e-2).
"""

import sys
import types

for _p in ("/root/.axon_site/_ro/trn_rl_repo", "/opt/trn_rl_repo"):
    if _p not in sys.path:
        sys.path.append(_p)

import numpy as np
from contextlib import ExitStack

import concourse.bass as bass
import concourse.tile as tile
from concourse import mybir, bass_utils
from concourse._compat import with_exitstack
from concourse.vector_clock import ScopedClock

# ---------------------------------------------------------------------------
# Workarounds for this image.
# ---------------------------------------------------------------------------

# walrus on this image rejects >1 sync-wait on one CTRL instruction; split the
# Tile exit-drain's waits across follow-up nops (same engine, program order).
# Also: the stock tail (barrier + per-sem clear + barrier) costs ~8-10us of
# EVSEM butterfly at kernel end. The kernel is one-shot per NEFF execution and
# NRT re-initialises semaphores per execution, so keep only the drain + DMA
# completion waits.
_ORIG_DAB = tile.TileContext._drain_and_barrier
_USE_STOCK_TAIL = False


def _patched_drain_and_barrier(self, tick_clock, wait_clock):
    if _USE_STOCK_TAIL:
        return _ORIG_DAB(self, tick_clock, wait_clock)
    nc = self.nc
    drain_inst = nc.sync.drain()
    wait_clock.add_sem_waits(
        drain_inst.ins, ScopedClock({None: tick_clock.global_clock})
    )
    si = drain_inst.ins.sync_info
    waits = list(si.on_wait) if si and si.on_wait else []
    if len(waits) > 1:
        drain_inst.ins.sync_info = mybir.SyncInfo(
            on_wait=[waits[0]], on_update=list(si.on_update or [])
        )
        for w in waits[1:]:
            nop = nc.sync.nop(nofuse=True, hint="drain_wait_split")
            nop.ins.sync_info = mybir.SyncInfo(on_wait=[w], on_update=[])
    popped = nc._tile_sem_poison_stack.pop()
    assert popped is self._sem_poison


tile.TileContext._drain_and_barrier = _patched_drain_and_barrier

# zero-egress container: profiling artifact upload must stay local.
bass_utils.upload_artifacts = lambda d: d


def _strip_main_barrier(nc):
    """Drop the prologue all-engine barrier AND the dead const memsets in
    'main': the barrier's only role is to fence the framework preamble (dead
    const memsets + per-engine table loads) from the kernel, but per-engine
    program order already covers the table loads, and nothing reads the
    const tiles (verified: no instruction references const-* memrefs).  Each
    engine then branches into the kernel as soon as its own init finishes
    instead of waiting for the slowest engine (~1.4us of startup)."""
    for fn in nc.m.functions:
        for bb in fn.blocks:
            if bb.name != "main":
                continue
            bb.instructions[:] = [
                i for i in bb.instructions
                if not isinstance(
                    i,
                    (mybir.InstEventSemaphore, mybir.InstDrain, mybir.InstMemset),
                )
            ]


def _split_multi_waits(nc):
    """walrus on this image rejects >1 sync-wait per instruction: hoist extra
    waits onto fresh NoOps inserted just before, on the same engine."""
    for fn in nc.m.functions:
        for bb in fn.blocks:
            new_insts = []
            for inst in bb.instructions:
                si = inst.sync_info
                waits = list(si.on_wait) if si and si.on_wait else []
                if len(waits) > 1:
                    for w in waits[:-1]:
                        nop = mybir.InstNoOp(
                            name=nc.get_next_instruction_name(),
                            sync_info=mybir.SyncInfo(on_wait=[w], on_update=[]),
                            bass_nofuse=True,
                            engine=inst.engine,
                        )
                        new_insts.append(nop)
                    inst.sync_info = mybir.SyncInfo(
                        on_wait=[waits[-1]], on_update=list(si.on_update or [])
                    )
                new_insts.append(inst)
            bb.instructions[:] = new_insts

# ---------------------------------------------------------------------------
# Problem constants (hardcoded; kernel.py must be self-contained).
# ---------------------------------------------------------------------------

B, C, H, W = 256, 3, 256, 256
LOW_A = LOW_B = 8
N_CORES = 8
IMGS_PER_CORE = (B // N_CORES) * C          # 96
GRP = 8                                     # images per main group (2 MiB f32)
N_MAIN = IMGS_PER_CORE // GRP - 1           # 11 main groups; 2x4-img tails
GROWS = GRP * H                             # 2048 rows per main group
SHED_ROWS = 16                              # rows per partition (16 KiB chunks)
TOTAL_LOW = B * C * LOW_A * LOW_B           # 49152 -> mean divisor

F32 = mybir.dt.float32
BF16 = mybir.dt.bfloat16


def _dct_basis(K, N):
    n = np.arange(N, dtype=np.float64)
    k = np.arange(K, dtype=np.float64)
    return (2.0 * np.cos(np.pi * (2.0 * n[None, :] + 1.0) * k[:, None] / (2.0 * N))).astype(
        np.float32
    )


def _make_consts():
    Ch = _dct_basis(LOW_A, H)   # [8, 256]
    Cw = _dct_basis(LOW_B, W)   # [8, 256]
    # Shed layout weights: partition p < 120 holds group rows 17p..17p+16,
    # partitions 120..127 hold rows 2040..2047 (one each).  Global row
    # g = 256q + h -> out row 8q+i gets Ch[i, h].
    wa17 = np.zeros((128, SHED_ROWS, 64), np.float32)
    for p in range(128):
        for r in range(SHED_ROWS):
            g = SHED_ROWS * p + r
            q, h = divmod(g, H)
            wa17[p, r, 8 * q:8 * q + 8] = Ch[:, h]
    # 4-image tail group: image q = p//32, rows h = 8*(p%32) + r.
    wa4 = np.zeros((128, 8, 32), np.float32)
    for p in range(128):
        q, pp = p // 32, p % 32
        for r in range(8):
            wa4[p, r, 8 * q:8 * q + 8] = Ch[:, 8 * pp + r]
    # 2-image tail groups: image q = p//64, rows h = 4*(p%64) + r.
    wa2 = np.zeros((128, 4, 16), np.float32)
    for p in range(128):
        q, pp = p // 64, p % 64
        for r in range(4):
            wa2[p, r, 8 * q:8 * q + 8] = Ch[:, 4 * pp + r]
    # cwt[p, wc, j] = Cw[j, wc*128+p]
    cwt = np.zeros((128, 2, LOW_B), np.float32)
    for wc in range(2):
        cwt[:, wc, :] = Cw[:, wc * 128:(wc + 1) * 128].T
    import ml_dtypes
    bf16 = ml_dtypes.bfloat16
    ident = np.eye(128, dtype=bf16)
    return wa17.astype(bf16), wa4.astype(bf16), wa2.astype(bf16), cwt.astype(bf16), ident


WA17, WA4, WA2, CWT, IDENT = _make_consts()


# ---------------------------------------------------------------------------
# Kernel body (per core; SPMD over 8 cores).
# ---------------------------------------------------------------------------

@with_exitstack
def _lowfreq_kernel(ctx: ExitStack, tc, out_ap, delta_ap, wa17_ap, wa4_ap,
                    wa2_ap, cwt_ap, ident_ap):
    nc = tc.nc

    const_pool = ctx.enter_context(tc.tile_pool(name="const", bufs=1))
    in8_pool = ctx.enter_context(tc.tile_pool(name="in8", bufs=N_MAIN))
    in4_pool = ctx.enter_context(tc.tile_pool(name="in4", bufs=2))
    sS_pool = ctx.enter_context(tc.tile_pool(name="sS", bufs=3))
    tS_pool = ctx.enter_context(tc.tile_pool(name="tS", bufs=3))
    red_pool = ctx.enter_context(tc.tile_pool(name="red", bufs=2))
    acc_pool = ctx.enter_context(tc.tile_pool(name="acc", bufs=1))
    psA_pool = ctx.enter_context(tc.tile_pool(name="psA", bufs=3, space="PSUM"))
    psT_pool = ctx.enter_context(tc.tile_pool(name="psT", bufs=3, space="PSUM"))
    ps2_pool = ctx.enter_context(tc.tile_pool(name="ps2", bufs=2, space="PSUM"))

    # constants (HWDGE/Sync queue; lands well before first compute)
    wa17 = const_pool.tile([128, SHED_ROWS, 64], BF16)
    nc.sync.dma_start(wa17[:], wa17_ap)
    wa4 = const_pool.tile([128, 8, 32], BF16)
    nc.sync.dma_start(wa4[:], wa4_ap)
    wa2 = const_pool.tile([128, 4, 16], BF16)
    nc.sync.dma_start(wa2[:], wa2_ap)
    cwt = const_pool.tile([128, 2, LOW_B], BF16)
    nc.sync.dma_start(cwt[:], cwt_ap)
    ident = const_pool.tile([128, 128], BF16)
    nc.sync.dma_start(ident[:], ident_ap)

    acc = acc_pool.tile([8, 1], F32)
    nc.vector.memset(acc[:], 0.0)

    # issue ALL input DMAs upfront (SWDGE, f32->bf16 inline cast).
    subs = []
    for g in range(N_MAIN):
        gt = in8_pool.tile([128, SHED_ROWS, 256], BF16, tag="gt8")
        fl = delta_ap[GRP * g:GRP * g + GRP].rearrange("q h w -> (q h) w")
        nc.gpsimd.dma_start(
            gt[:],
            fl.rearrange("(p r) w -> p (r w)", p=128, r=SHED_ROWS),
        )
        subs.append((gt, "main", GRP))
    gt4 = in4_pool.tile([128, 8, 256], BF16, tag="gt4")
    src = delta_ap[88:92]
    nc.gpsimd.dma_start(
        gt4[:],
        src.rearrange("q (pp r) w -> (q pp) (r w)", pp=32, r=8),
    )
    subs.append((gt4, "tail", 4))
    for t in range(2):
        gt2 = in4_pool.tile([128, 4, 256], BF16, tag="gt2")
        src = delta_ap[92 + 2 * t:94 + 2 * t]
        nc.gpsimd.dma_start(
            gt2[:],
            src.rearrange("q (pp r) w -> (q pp) (r w)", pp=64, r=4),
        )
        subs.append((gt2, "tail", 2))

    def stage_a(sub):
        gt, kind, n_img = sub
        n_out = 8 * n_img
        psumA = psA_pool.tile([n_out, 256], F32, tag="psA")
        wA = {8: wa17, 4: wa4, 2: wa2}[n_img]
        rows = gt.shape[1]
        for r in range(rows):
            nc.tensor.matmul(
                psumA[:], lhsT=wA[:, r, :], rhs=gt[:, r, :],
                start=(r == 0), stop=(r == rows - 1),
            )
        # PSUM -> SBUF with f32->bf16 cast (ACT engine; off the PE
        # timeline).  Two half tiles so stage B's first transpose only
        # waits on the first half (shortens the post-stream tail).
        sAs = []
        for wc in range(2):
            sA = sS_pool.tile([n_out, 128], BF16, tag=f"sA{wc}")
            nc.scalar.copy(sA[:], psumA[:, 128 * wc:128 * wc + 128])
            sAs.append(sA)
        return sAs, n_out

    def stage_b(sAs, n_out):
        # 2 PE transposes (own PSUM tiles: transpose-mode output must start
        # at a bank boundary on HW), DVE copies out, then contract w into
        # ps2[j, (q,i)], fused |.|+sum, accumulate.
        tS = tS_pool.tile([128, 2, n_out], BF16, tag="tS")
        for wc in range(2):
            tp = psT_pool.tile([128, n_out], BF16, tag="tp")
            nc.tensor.transpose(
                tp[:],
                sAs[wc][:],
                ident[0:n_out, 0:n_out],
            )
            nc.vector.tensor_copy(tS[:, wc, :], tp[:])
        ps2 = ps2_pool.tile([8, n_out], F32, tag="ps2")
        for wc in range(2):
            nc.tensor.matmul(
                ps2[:],
                lhsT=cwt[:, wc, :],
                rhs=tS[:, wc, :],
                start=(wc == 0),
                stop=(wc == 1),
            )
        red = red_pool.tile([8, 1], F32)
        nc.vector.tensor_reduce(
            red[:], ps2[:], axis=mybir.AxisListType.X,
            op=mybir.AluOpType.add, apply_absolute_value=True,
        )
        nc.vector.tensor_add(acc[:], acc[:], red[:])

    # Software pipeline: emit stage B of group g-1 AFTER stage A of group g,
    # so the PE (which executes in program order) never stalls mid-stream on
    # the ACT/DVE round-trips of stage B.
    prev = None
    for sub in subs:
        cur = stage_a(sub)
        if prev is not None:
            stage_b(*prev)
        prev = cur
    stage_b(*prev)

    # ship the 8 per-partition partials; the host does the final sum + mean.
    # Issue on Scalar (HWDGE) so it overlaps the SP exit-drain sem walk.
    nc.scalar.dma_start(out_ap, acc[:])


# ---------------------------------------------------------------------------
# Build + run.
# ---------------------------------------------------------------------------

_CACHED_NC = None


def _build(for_sim=False):
    global _CACHED_NC, _USE_STOCK_TAIL
    if not for_sim and _CACHED_NC is not None:
        return _CACHED_NC
    _USE_STOCK_TAIL = for_sim
    nc = bass.Bass("TRN2", target_bir_lowering=False, debug=False)
    delta = nc.dram_tensor("delta", [IMGS_PER_CORE, H, W], F32, kind="ExternalInput")
    wa17 = nc.dram_tensor("wa17", list(WA17.shape), BF16, kind="ExternalInput")
    wa4 = nc.dram_tensor("wa4", list(WA4.shape), BF16, kind="ExternalInput")
    wa2 = nc.dram_tensor("wa2", list(WA2.shape), BF16, kind="ExternalInput")
    cwt = nc.dram_tensor("cwt", list(CWT.shape), BF16, kind="ExternalInput")
    ident = nc.dram_tensor("ident", list(IDENT.shape), BF16, kind="ExternalInput")
    out = nc.dram_tensor("out", [8, 1], F32, kind="ExternalOutput")

    with tile.TileContext(nc) as tc:
        _lowfreq_kernel(
            tc, out.ap(), delta.ap(), wa17.ap(), wa4.ap(), wa2.ap(), cwt.ap(),
            ident.ap()
        )
    _USE_STOCK_TAIL = False
    if for_sim:
        return nc
    _strip_main_barrier(nc)
    _split_multi_waits(nc)
    _CACHED_NC = nc
    return nc


def _run(delta, **spmd_kwargs):
    import os
    os.environ["JAX_PLATFORMS"] = "axon"   # harness may have pinned cpu for the reference
    nc = _build()
    delta = np.ascontiguousarray(np.asarray(delta, dtype=np.float32))
    assert delta.shape == (B, C, H, W)
    shards = delta.reshape(N_CORES, IMGS_PER_CORE, H, W)
    in_maps = [
        {
            "delta": shards[i],
            "wa17": WA17,
            "wa4": WA4,
            "wa2": WA2,
            "cwt": CWT,
            "ident": IDENT,
        }
        for i in range(N_CORES)
    ]
    try:
        res = bass_utils.run_bass_kernel_spmd(
            nc, in_maps, core_ids=list(range(N_CORES)), **spmd_kwargs
        )
    except Exception:
        # transient NRT_EXEC_UNIT_UNRECOVERABLE has been observed on this
        # terminal; one retry typically succeeds.
        res = bass_utils.run_bass_kernel_spmd(
            nc, in_maps, core_ids=list(range(N_CORES)), **spmd_kwargs
        )
    total = np.float64(0.0)
    for r in res.results:
        total += np.asarray(r["out"], np.float64).sum()
    return np.float32(total / TOTAL_LOW).reshape(()), res


def kernel(delta):
    out, _ = _run(delta)
    return out


# revision 22
# speedup vs baseline: 1.2875x; 1.1686x over previous
"""Trainium2 Bass kernel for nn_LowFreqPenaltyLoss.

Computes mean(|einsum('ih,nchw,jw->ncij', Ch, delta, Cw)|) for
delta [256, 3, 256, 256] f32, Ch/Cw the 8x256 unnormalized DCT-II bases.

Strategy (data-parallel over batch, 8 cores), per core 96 images = 24 MiB:

  LOAD (the problem is HBM-stream-bound): 11 groups of 8 images (2 MiB)
  + 2 tail groups of 4 images (1 MiB), all via SWDGE DMAs that cast
  f32->bf16 inline.  Partition p receives a CONTIGUOUS HBM chunk (16-17 KiB)
  so descriptors/packets are full-sized (the old per-row 1 KiB gather
  streamed at ~275 GB/s read-side; flat chunks reach ~400-410 GB/s).

  ENGINE-15 SHED: SDMA engine 15 is intermittently ~15-25% slower per
  packet (known SWDGE descriptor-ring port contention); in those runs it
  alone stretches the stream by ~10 us (engines are partition-pinned, no
  work stealing).  Main groups therefore give partitions 120-127 only 1 of
  the 2048 group rows (two rectangular DMAs: [0:120]x17 rows + [120:128]x1
  row), halving engine 15's and 13's load at +6% on the rest: the slow mode
  stops being the long pole at a ~2 us cost in the fast mode.

  STAGE A (contract h): each partition's rows sit on the free axis, so the
  DCT-H contraction is 17 accumulating matmuls with block-sparse weights
  wa17[p, r, 8q+i] = Ch[i, h] for global row 17p+r = 256q + h (r=0 uses all
  128 partitions, r>=1 only [0:120)) -> psumA[8q+i, w] f32.

  STAGE B (contract w): ACT copies psumA -> SBUF (casts bf16), 2 PE
  transposes of [64,128] chunks (each into its own PSUM bank), DVE copies
  out, matmul with CwT -> ps2[j, (q,i)], fused |.|+sum on DVE into acc[8,1].
  Stage B of group g-1 is emitted AFTER stage A of group g so the PE (which
  runs in program order) never stalls mid-stream on ACT/DVE round-trips.

  FINISH: the 8 per-partition partials ship via a Scalar-engine DMA (it
  overlaps the SP exit-drain sem walk); the host sums 8 cores x 8 partials
  and divides by 49152.  bf16 inputs + f32 PSUM accumulation give ~2e-4
  relative error on the final scalar (gate is 2e-2).
"""

import sys
import types

for _p in ("/root/.axon_site/_ro/trn_rl_repo", "/opt/trn_rl_repo"):
    if _p not in sys.path:
        sys.path.append(_p)

import numpy as np
from contextlib import ExitStack

import concourse.bass as bass
import concourse.tile as tile
from concourse import mybir, bass_utils
from concourse._compat import with_exitstack
from concourse.vector_clock import ScopedClock

# ---------------------------------------------------------------------------
# Workarounds for this image.
# ---------------------------------------------------------------------------

# walrus on this image rejects >1 sync-wait on one CTRL instruction; split the
# Tile exit-drain's waits across follow-up nops (same engine, program order).
# Also: the stock tail (barrier + per-sem clear + barrier) costs ~8-10us of
# EVSEM butterfly at kernel end. The kernel is one-shot per NEFF execution and
# NRT re-initialises semaphores per execution, so keep only the drain + DMA
# completion waits.
_ORIG_DAB = tile.TileContext._drain_and_barrier
_USE_STOCK_TAIL = False


def _patched_drain_and_barrier(self, tick_clock, wait_clock):
    if _USE_STOCK_TAIL:
        return _ORIG_DAB(self, tick_clock, wait_clock)
    nc = self.nc
    drain_inst = nc.sync.drain()
    wait_clock.add_sem_waits(
        drain_inst.ins, ScopedClock({None: tick_clock.global_clock})
    )
    si = drain_inst.ins.sync_info
    waits = list(si.on_wait) if si and si.on_wait else []
    if len(waits) > 1:
        drain_inst.ins.sync_info = mybir.SyncInfo(
            on_wait=[waits[0]], on_update=list(si.on_update or [])
        )
        for w in waits[1:]:
            nop = nc.sync.nop(nofuse=True, hint="drain_wait_split")
            nop.ins.sync_info = mybir.SyncInfo(on_wait=[w], on_update=[])
    popped = nc._tile_sem_poison_stack.pop()
    assert popped is self._sem_poison


tile.TileContext._drain_and_barrier = _patched_drain_and_barrier

# zero-egress container: profiling artifact upload must stay local.
bass_utils.upload_artifacts = lambda d: d


def _strip_main_barrier(nc):
    """Drop the prologue all-engine barrier AND the dead const memsets in
    'main': the barrier's only role is to fence the framework preamble (dead
    const memsets + per-engine table loads) from the kernel, but per-engine
    program order already covers the table loads, and nothing reads the
    const tiles (verified: no instruction references const-* memrefs).  Each
    engine then branches into the kernel as soon as its own init finishes
    instead of waiting for the slowest engine (~1.4us of startup)."""
    for fn in nc.m.functions:
        for bb in fn.blocks:
            if bb.name != "main":
                continue
            bb.instructions[:] = [
                i for i in bb.instructions
                if not isinstance(
                    i,
                    (mybir.InstEventSemaphore, mybir.InstDrain, mybir.InstMemset),
                )
            ]


def _split_multi_waits(nc):
    """walrus on this image rejects >1 sync-wait per instruction: hoist extra
    waits onto fresh NoOps inserted just before, on the same engine."""
    for fn in nc.m.functions:
        for bb in fn.blocks:
            new_insts = []
            for inst in bb.instructions:
                si = inst.sync_info
                waits = list(si.on_wait) if si and si.on_wait else []
                if len(waits) > 1:
                    for w in waits[:-1]:
                        nop = mybir.InstNoOp(
                            name=nc.get_next_instruction_name(),
                            sync_info=mybir.SyncInfo(on_wait=[w], on_update=[]),
                            bass_nofuse=True,
                            engine=inst.engine,
                        )
                        new_insts.append(nop)
                    inst.sync_info = mybir.SyncInfo(
                        on_wait=[waits[-1]], on_update=list(si.on_update or [])
                    )
                new_insts.append(inst)
            bb.instructions[:] = new_insts

# ---------------------------------------------------------------------------
# Problem constants (hardcoded; kernel.py must be self-contained).
# ---------------------------------------------------------------------------

B, C, H, W = 256, 3, 256, 256
LOW_A = LOW_B = 8
N_CORES = 8
IMGS_PER_CORE = (B // N_CORES) * C          # 96
GRP = 8                                     # images per main group (2 MiB f32)
N_MAIN = IMGS_PER_CORE // GRP - 1           # 11 main groups; 2x4-img tails
GROWS = GRP * H                             # 2048 rows per main group
SHED_ROWS = 16                              # rows per partition (16 KiB chunks)
TOTAL_LOW = B * C * LOW_A * LOW_B           # 49152 -> mean divisor

F32 = mybir.dt.float32
BF16 = mybir.dt.bfloat16


def _dct_basis(K, N):
    n = np.arange(N, dtype=np.float64)
    k = np.arange(K, dtype=np.float64)
    return (2.0 * np.cos(np.pi * (2.0 * n[None, :] + 1.0) * k[:, None] / (2.0 * N))).astype(
        np.float32
    )


def _make_consts():
    Ch = _dct_basis(LOW_A, H)   # [8, 256]
    Cw = _dct_basis(LOW_B, W)   # [8, 256]
    # Shed layout weights: partition p < 120 holds group rows 17p..17p+16,
    # partitions 120..127 hold rows 2040..2047 (one each).  Global row
    # g = 256q + h -> out row 8q+i gets Ch[i, h].
    wa17 = np.zeros((128, SHED_ROWS, 64), np.float32)
    for p in range(128):
        for r in range(SHED_ROWS):
            g = SHED_ROWS * p + r
            q, h = divmod(g, H)
            wa17[p, r, 8 * q:8 * q + 8] = Ch[:, h]
    # 4-image tail group: image q = p//32, rows h = 8*(p%32) + r.
    wa4 = np.zeros((128, 8, 32), np.float32)
    for p in range(128):
        q, pp = p // 32, p % 32
        for r in range(8):
            wa4[p, r, 8 * q:8 * q + 8] = Ch[:, 8 * pp + r]
    # 2-image tail groups: image q = p//64, rows h = 4*(p%64) + r.
    wa2 = np.zeros((128, 4, 16), np.float32)
    for p in range(128):
        q, pp = p // 64, p % 64
        for r in range(4):
            wa2[p, r, 8 * q:8 * q + 8] = Ch[:, 4 * pp + r]
    # cwt[p, wc, j] = Cw[j, wc*128+p]
    cwt = np.zeros((128, 2, LOW_B), np.float32)
    for wc in range(2):
        cwt[:, wc, :] = Cw[:, wc * 128:(wc + 1) * 128].T
    import ml_dtypes
    bf16 = ml_dtypes.bfloat16
    ident = np.eye(128, dtype=bf16)
    return wa17.astype(bf16), wa4.astype(bf16), wa2.astype(bf16), cwt.astype(bf16), ident


WA17, WA4, WA2, CWT, IDENT = _make_consts()


# ---------------------------------------------------------------------------
# Kernel body (per core; SPMD over 8 cores).
# ---------------------------------------------------------------------------

@with_exitstack
def _lowfreq_kernel(ctx: ExitStack, tc, out_ap, delta_ap, wa17_ap, wa4_ap,
                    wa2_ap, cwt_ap, ident_ap):
    nc = tc.nc

    const_pool = ctx.enter_context(tc.tile_pool(name="const", bufs=1))
    in8_pool = ctx.enter_context(tc.tile_pool(name="in8", bufs=N_MAIN))
    in4_pool = ctx.enter_context(tc.tile_pool(name="in4", bufs=2))
    sS_pool = ctx.enter_context(tc.tile_pool(name="sS", bufs=3))
    tS_pool = ctx.enter_context(tc.tile_pool(name="tS", bufs=3))
    red_pool = ctx.enter_context(tc.tile_pool(name="red", bufs=2))
    acc_pool = ctx.enter_context(tc.tile_pool(name="acc", bufs=1))
    psA_pool = ctx.enter_context(tc.tile_pool(name="psA", bufs=3, space="PSUM"))
    psT_pool = ctx.enter_context(tc.tile_pool(name="psT", bufs=3, space="PSUM"))
    ps2_pool = ctx.enter_context(tc.tile_pool(name="ps2", bufs=2, space="PSUM"))

    # constants (HWDGE/Sync queue; lands well before first compute)
    wa17 = const_pool.tile([128, SHED_ROWS, 64], BF16)
    nc.sync.dma_start(wa17[:], wa17_ap)
    wa4 = const_pool.tile([128, 8, 32], BF16)
    nc.sync.dma_start(wa4[:], wa4_ap)
    wa2 = const_pool.tile([128, 4, 16], BF16)
    nc.sync.dma_start(wa2[:], wa2_ap)
    cwt = const_pool.tile([128, 2, LOW_B], BF16)
    nc.sync.dma_start(cwt[:], cwt_ap)
    # ident via the Scalar HWDGE ring: warms qScalarDynamicHW so the final
    # out-DMA (also Scalar) doesn't pay first-use ring setup (~0.6us).
    ident = const_pool.tile([128, 128], BF16)
    nc.scalar.dma_start(ident[:], ident_ap)

    acc = acc_pool.tile([8, 1], F32)
    nc.vector.memset(acc[:], 0.0)

    # issue ALL input DMAs upfront (SWDGE, f32->bf16 inline cast).
    subs = []
    for g in range(N_MAIN):
        gt = in8_pool.tile([128, SHED_ROWS, 256], BF16, tag="gt8")
        fl = delta_ap[GRP * g:GRP * g + GRP].rearrange("q h w -> (q h) w")
        nc.gpsimd.dma_start(
            gt[:],
            fl.rearrange("(p r) w -> p (r w)", p=128, r=SHED_ROWS),
        )
        subs.append((gt, "main", GRP))
    gt4 = in4_pool.tile([128, 8, 256], BF16, tag="gt4")
    src = delta_ap[88:92]
    nc.gpsimd.dma_start(
        gt4[:],
        src.rearrange("q (pp r) w -> (q pp) (r w)", pp=32, r=8),
    )
    subs.append((gt4, "tail", 4))
    for t in range(2):
        gt2 = in4_pool.tile([128, 4, 256], BF16, tag="gt2")
        src = delta_ap[92 + 2 * t:94 + 2 * t]
        nc.gpsimd.dma_start(
            gt2[:],
            src.rearrange("q (pp r) w -> (q pp) (r w)", pp=64, r=4),
        )
        subs.append((gt2, "tail", 2))

    def stage_a(sub):
        gt, kind, n_img = sub
        n_out = 8 * n_img
        psumA = psA_pool.tile([n_out, 256], F32, tag="psA")
        wA = {8: wa17, 4: wa4, 2: wa2}[n_img]
        rows = gt.shape[1]
        for r in range(rows):
            nc.tensor.matmul(
                psumA[:], lhsT=wA[:, r, :], rhs=gt[:, r, :],
                start=(r == 0), stop=(r == rows - 1),
            )
        # PSUM -> SBUF with f32->bf16 cast (ACT engine; off the PE
        # timeline).  Two half tiles so stage B's first transpose only
        # waits on the first half (shortens the post-stream tail).
        sAs = []
        for wc in range(2):
            sA = sS_pool.tile([n_out, 128], BF16, tag=f"sA{wc}")
            nc.scalar.copy(sA[:], psumA[:, 128 * wc:128 * wc + 128])
            sAs.append(sA)
        return sAs, n_out

    def stage_b(sAs, n_out):
        # 2 PE transposes (own PSUM tiles: transpose-mode output must start
        # at a bank boundary on HW), DVE copies out, then contract w into
        # ps2[j, (q,i)], fused |.|+sum, accumulate.
        tS = tS_pool.tile([128, 2, n_out], BF16, tag="tS")
        for wc in range(2):
            tp = psT_pool.tile([128, n_out], BF16, tag="tp")
            nc.tensor.transpose(
                tp[:],
                sAs[wc][:],
                ident[0:n_out, 0:n_out],
            )
            nc.vector.tensor_copy(tS[:, wc, :], tp[:])
        ps2 = ps2_pool.tile([8, n_out], F32, tag="ps2")
        for wc in range(2):
            nc.tensor.matmul(
                ps2[:],
                lhsT=cwt[:, wc, :],
                rhs=tS[:, wc, :],
                start=(wc == 0),
                stop=(wc == 1),
            )
        red = red_pool.tile([8, 1], F32)
        nc.vector.tensor_reduce(
            red[:], ps2[:], axis=mybir.AxisListType.X,
            op=mybir.AluOpType.add, apply_absolute_value=True,
        )
        nc.vector.tensor_add(acc[:], acc[:], red[:])

    # Software pipeline: emit stage B of group g-1 AFTER stage A of group g,
    # so the PE (which executes in program order) never stalls mid-stream on
    # the ACT/DVE round-trips of stage B.
    prev = None
    for sub in subs:
        cur = stage_a(sub)
        if prev is not None:
            stage_b(*prev)
        prev = cur
    stage_b(*prev)

    # ship the 8 per-partition partials; the host does the final sum + mean.
    # Issue on Scalar (HWDGE) so it overlaps the SP exit-drain sem walk.
    nc.scalar.dma_start(out_ap, acc[:])


# ---------------------------------------------------------------------------
# Build + run.
# ---------------------------------------------------------------------------

_CACHED_NC = None


def _build(for_sim=False):
    global _CACHED_NC, _USE_STOCK_TAIL
    if not for_sim and _CACHED_NC is not None:
        return _CACHED_NC
    _USE_STOCK_TAIL = for_sim
    nc = bass.Bass("TRN2", target_bir_lowering=False, debug=False)
    delta = nc.dram_tensor("delta", [IMGS_PER_CORE, H, W], F32, kind="ExternalInput")
    wa17 = nc.dram_tensor("wa17", list(WA17.shape), BF16, kind="ExternalInput")
    wa4 = nc.dram_tensor("wa4", list(WA4.shape), BF16, kind="ExternalInput")
    wa2 = nc.dram_tensor("wa2", list(WA2.shape), BF16, kind="ExternalInput")
    cwt = nc.dram_tensor("cwt", list(CWT.shape), BF16, kind="ExternalInput")
    ident = nc.dram_tensor("ident", list(IDENT.shape), BF16, kind="ExternalInput")
    out = nc.dram_tensor("out", [8, 1], F32, kind="ExternalOutput")

    with tile.TileContext(nc) as tc:
        _lowfreq_kernel(
            tc, out.ap(), delta.ap(), wa17.ap(), wa4.ap(), wa2.ap(), cwt.ap(),
            ident.ap()
        )
    _USE_STOCK_TAIL = False
    if for_sim:
        return nc
    _strip_main_barrier(nc)
    _split_multi_waits(nc)
    _CACHED_NC = nc
    return nc


def _run(delta, **spmd_kwargs):
    import os
    os.environ["JAX_PLATFORMS"] = "axon"   # harness may have pinned cpu for the reference
    nc = _build()
    delta = np.ascontiguousarray(np.asarray(delta, dtype=np.float32))
    assert delta.shape == (B, C, H, W)
    shards = delta.reshape(N_CORES, IMGS_PER_CORE, H, W)
    in_maps = [
        {
            "delta": shards[i],
            "wa17": WA17,
            "wa4": WA4,
            "wa2": WA2,
            "cwt": CWT,
            "ident": IDENT,
        }
        for i in range(N_CORES)
    ]
    try:
        res = bass_utils.run_bass_kernel_spmd(
            nc, in_maps, core_ids=list(range(N_CORES)), **spmd_kwargs
        )
    except Exception:
        # transient NRT_EXEC_UNIT_UNRECOVERABLE has been observed on this
        # terminal; one retry typically succeeds.
        res = bass_utils.run_bass_kernel_spmd(
            nc, in_maps, core_ids=list(range(N_CORES)), **spmd_kwargs
        )
    total = np.float64(0.0)
    for r in res.results:
        total += np.asarray(r["out"], np.float64).sum()
    return np.float32(total / TOTAL_LOW).reshape(()), res


def kernel(delta):
    out, _ = _run(delta)
    return out
